# revision 9
# baseline (speedup 1.0000x reference)
"""Trainium2 Bass kernel for nn_PoolNU: gather + max-pool over neighbour table.

reference:
    x: (8, 128, 65536) f32, neighbours: (9, 16384) int
    out[b, c, j] = max_k x[b, c, neighbours[k, j]]

Strategy:
    - The neighbour table is shared across (b, c), so one gathered "row" can
      carry ALL batches and channels for a location. Host repacks x to
      x_merged (65536, B*C=1024) — one 4KB row per location. This makes each
      gathered descriptor 4KB instead of 512B: 8x fewer descriptors, which
      matters because the gpsimd dma_gather ucode generates descriptors at
      only ~6-8 ns each.
    - Output locations (16384) are sharded across the 8 NeuronCores (2048
      per core). Each core needs at most 9*2048=18432 distinct source rows,
      which the host compacts into a per-core x_sub with remapped indices —
      guaranteed to fit dma_gather's int16 index window (< 32768), so no
      window splitting is needed at all.
    - Device per tile of 128 locations: gather 9*128 rows (two <=1024-index
      dma_gather calls), vector max-reduce over the 9 slots, store 4KB rows.
    - Host reassembles (core, loc, b, c) -> (b, c, loc).
"""

import sys

sys.path.insert(0, "/opt/trn_rl_repo")

import hashlib

import numpy as np

import concourse.mybir as mybir
from concourse import bacc, bass_utils
from concourse.tile import TileContext

B = 8
C = 128
LIN = 65536
K = 9
LOUT = 16384

P = 128
NCORE = 8
LPC = LOUT // NCORE          # locations per core (2048)
NTILE = LPC // P             # tiles per core (16)
E = B * C                    # elements per gathered row (1024)
UMAX = K * LPC               # padded x_sub rows (18432)
NMAX = 1024                  # max indices per dma_gather call

_CACHE = {}


def _build_program():
    nc = bacc.Bacc("TRN2", target_bir_lowering=False, debug=False, num_devices=1)

    xs = nc.dram_tensor("xs", [UMAX, E], mybir.dt.float32, kind="ExternalInput")
    # idx per tile: two calls (8 slots then 1 slot), each 16-wrapped and
    # replicated over the 128 partitions in groups of 16.
    WA, WB = NMAX // 16, P // 16
    WT = WA + WB
    idx = nc.dram_tensor("idx", [P, NTILE * WT], mybir.dt.int16,
                         kind="ExternalInput")
    out = nc.dram_tensor("out", [LPC, E], mybir.dt.float32, kind="ExternalOutput")

    with TileContext(nc) as tc:
        with tc.tile_pool(name="sbuf", bufs=3) as pool:
            idx_sb = pool.tile([P, NTILE * WT], mybir.dt.int16, bufs=1)
            nc.sync.dma_start(out=idx_sb[:], in_=idx.ap())

            for t in range(NTILE):
                g = pool.tile([P, K * E], mybir.dt.float32, tag="g")
                c0 = t * WT
                nc.gpsimd.dma_gather(
                    out_ap=g[:, : 8 * E].rearrange("p (g e) -> p g e", e=E),
                    in_ap=xs.ap(),
                    idxs_ap=idx_sb[:, c0 : c0 + WA],
                    num_idxs=NMAX,
                    num_idxs_reg=NMAX,
                    elem_size=E,
                )
                nc.gpsimd.dma_gather(
                    out_ap=g[:, 8 * E : K * E].rearrange("p (g e) -> p g e", e=E),
                    in_ap=xs.ap(),
                    idxs_ap=idx_sb[:, c0 + WA : c0 + WT],
                    num_idxs=P,
                    num_idxs_reg=P,
                    elem_size=E,
                )
                acc = pool.tile([P, E], mybir.dt.float32, tag="acc", bufs=3)
                nc.vector.tensor_tensor(
                    out=acc[:], in0=g[:, 0:E], in1=g[:, E : 2 * E],
                    op=mybir.AluOpType.max,
                )
                for s in range(2, K):
                    nc.vector.tensor_tensor(
                        out=acc[:], in0=acc[:], in1=g[:, s * E : (s + 1) * E],
                        op=mybir.AluOpType.max,
                    )
                nc.sync.dma_start(out=out.ap()[t * P : (t + 1) * P, :], in_=acc[:])

    nc.compile()
    return nc


def _get_program():
    if "nc" not in _CACHE:
        _CACHE["nc"] = _build_program()
    return _CACHE["nc"]


def _wrap16(lst: np.ndarray) -> np.ndarray:
    """(N,) int -> (128, N/16) int16: 16-partition wrap, replicated x8."""
    w = len(lst) // 16
    return np.tile(lst.reshape(w, 16).T, (8, 1)).astype(np.int16)


def kernel(x: np.ndarray, neighbours: np.ndarray) -> np.ndarray:
    x = np.asarray(x)
    nb = np.asarray(neighbours).astype(np.int64)          # (K, LOUT)
    assert x.shape == (B, C, LIN) and x.dtype == np.float32
    assert nb.shape == (K, LOUT)

    # (LIN, B*C): one 4KB row per input location
    xm = np.ascontiguousarray(x.transpose(2, 0, 1).reshape(LIN, E))

    in_maps = []
    for core in range(NCORE):
        nbc = nb[:, core * LPC : (core + 1) * LPC]        # (K, LPC)
        uniq, inv = np.unique(nbc, return_inverse=True)
        inv = inv.reshape(K, LPC)
        xs = np.empty((UMAX, E), dtype=np.float32)
        xs[: len(uniq)] = xm[uniq]
        cols = []
        for t in range(NTILE):
            loc2d = inv[:, t * P : (t + 1) * P]           # (K, P) local idx
            # call A: slots 0..7 -> list[(s)*128+p] = loc2d[s, p]
            cols.append(_wrap16(loc2d[:8].ravel()))
            # call B: slot 8
            cols.append(_wrap16(loc2d[8].ravel()))
        idx_np = np.ascontiguousarray(np.concatenate(cols, axis=1))
        in_maps.append({"xs": xs, "idx": idx_np})

    nc = _get_program()
    res = bass_utils.run_bass_kernel_spmd(nc, in_maps, core_ids=list(range(NCORE)))
    _CACHE["last_result"] = res

    # out per core: (LPC, B*C) -> full (B, C, LOUT)
    dev = np.concatenate([res.results[c]["out"] for c in range(NCORE)])  # (LOUT, E)
    return np.ascontiguousarray(dev.reshape(LOUT, B, C).transpose(1, 2, 0))
